# revision 1
# baseline (speedup 1.0000x reference)
"""Trainium2 Bass kernel for nn_Classify1 (retrieval_knn).

Reference computation:
  pd[b,n,m] = 2*<x_bn, y_bm> - |x_bn|^2 - |y_bm|^2     (neg. sq. distance)
  dist      = top_k(pd, 20)                            (descending)
  out       = sigmoid(W3 @ relu(bn2(W2 @ relu(bn1(W1 @ dist^T)))))

Strategy: shard the B*N = 16384 query rows across 8 cores (2048 each; 4
cores per batch, y replicated per batch). Each core computes its
[2048, 8192] distance slab via an augmented K=8 matmul directly into PSUM
(the 536MB distance matrix never touches HBM), extracts top-20 per row
with DVE max8/match_replace, and runs the (BN-folded) MLP stack locally.
"""

import numpy as np

B, N, M, C = 2, 8192, 8192, 3
K = 20
N_CORES = 8
CORES_PER_BATCH = N_CORES // B
ROWS_PER_CORE = B * N // N_CORES          # 2048
RT = ROWS_PER_CORE // 128                 # 16 row-tiles of 128 queries
CHUNK = 512                               # PSUM bank = 512 f32
NCH = M // CHUNK                          # 16 chunks per row
KAUG = 8                                  # augmented contraction dim (5 used, padded)
BN_EPS = 1e-5
NEG_INF = -1e30

# Top-k candidate generation mode:
#   "exact512": per 512-chunk top-16 via (max8, match_replace, max8) — 3 DVE scans
#   "sub256":   per 256-subchunk top-8 via 2x max8 — 1 DVE scan
#   "sub128":   per 128-subchunk top-8 via 4x max8 — 1 DVE scan
TOPK_MODE = "exact512"

_CACHE = {}


def _cands_per_chunk(mode):
    return {"exact512": 16, "sub256": 16, "sub128": 32}[mode]


def _build(mode):
    import concourse.bacc as bacc
    import concourse.mybir as mybir
    import concourse.tile as tile
    from concourse.masks import make_identity

    f32 = mybir.dt.float32
    nc = bacc.Bacc(None, target_bir_lowering=False, name="knn_classify")

    xaug_d = nc.dram_tensor("xaug", [KAUG, ROWS_PER_CORE], f32, kind="ExternalInput")
    yaug_d = nc.dram_tensor("yaug", [KAUG, M], f32, kind="ExternalInput")
    w1t_d = nc.dram_tensor("w1t", [K, 256], f32, kind="ExternalInput")
    b1_d = nc.dram_tensor("b1", [128, 2], f32, kind="ExternalInput")
    w2t_d = nc.dram_tensor("w2t", [128, 2, 128], f32, kind="ExternalInput")
    b2_d = nc.dram_tensor("b2", [128, 1], f32, kind="ExternalInput")
    w3t_d = nc.dram_tensor("w3t", [128, 1], f32, kind="ExternalInput")
    out_d = nc.dram_tensor("out", [1, ROWS_PER_CORE], f32, kind="ExternalOutput")

    NCAND = NCH * _cands_per_chunk(mode)

    with tile.TileContext(nc) as tc:
        with (
            tc.tile_pool(name="const", bufs=1) as const_pool,
            tc.tile_pool(name="cand", bufs=3) as cand_pool,
            tc.tile_pool(name="psum_pd", bufs=4, space="PSUM") as psum_pd,
            tc.tile_pool(name="psum_t", bufs=2, space="PSUM") as psum_t,
            tc.tile_pool(name="psum_o", bufs=2, space="PSUM") as psum_o,
        ):
            # --- load constants / inputs ---
            xaug = const_pool.tile([KAUG, ROWS_PER_CORE], f32)
            nc.sync.dma_start(xaug[:], xaug_d[:])
            yaug = const_pool.tile([KAUG, M], f32)
            nc.sync.dma_start(yaug[:], yaug_d[:])
            w1t = const_pool.tile([K, 256], f32)
            nc.sync.dma_start(w1t[:], w1t_d[:])
            b1 = const_pool.tile([128, 2], f32)
            nc.sync.dma_start(b1[:], b1_d[:])
            w2t = const_pool.tile([128, 2, 128], f32)
            nc.sync.dma_start(w2t[:], w2t_d[:])
            b2 = const_pool.tile([128, 1], f32)
            nc.sync.dma_start(b2[:], b2_d[:])
            w3t = const_pool.tile([128, 1], f32)
            nc.sync.dma_start(w3t[:], w3t_d[:])
            identity = const_pool.tile([128, 128], f32)
            make_identity(nc, identity[:])

            feat = const_pool.tile([K, ROWS_PER_CORE], f32)   # top-20 dists, [20, n]
            h1 = const_pool.tile([128, 2, ROWS_PER_CORE], f32)
            h2 = const_pool.tile([128, ROWS_PER_CORE], f32)
            out_sb = const_pool.tile([1, ROWS_PER_CORE], f32)

            # --- distance + top-k per 128-row tile ---
            for rt in range(RT):
                lhs = xaug[:, rt * 128:(rt + 1) * 128]
                cand = cand_pool.tile([128, NCAND], f32, tag="cand")
                for ch in range(NCH):
                    ps = psum_pd.tile([128, CHUNK], f32, tag="pd")
                    nc.tensor.matmul(
                        ps[:], lhs, yaug[:, ch * CHUNK:(ch + 1) * CHUNK],
                        start=True, stop=True,
                    )
                    if mode == "exact512":
                        c0 = ch * 16
                        nc.vector.max(cand[:, c0:c0 + 8], ps[:])
                        nc.vector.match_replace(ps[:], cand[:, c0:c0 + 8], ps[:], NEG_INF)
                        nc.vector.max(cand[:, c0 + 8:c0 + 16], ps[:])
                    elif mode == "sub256":
                        for s in range(2):
                            c0 = (ch * 2 + s) * 8
                            nc.vector.max(cand[:, c0:c0 + 8], ps[:, s * 256:(s + 1) * 256])
                    elif mode == "sub128":
                        for s in range(4):
                            c0 = (ch * 4 + s) * 8
                            nc.vector.max(cand[:, c0:c0 + 8], ps[:, s * 128:(s + 1) * 128])

                # top-24 of the candidates (sorted desc); first 20 are the answer
                top = cand_pool.tile([128, 24], f32, tag="top")
                nc.vector.max(top[:, 0:8], cand[:])
                nc.vector.match_replace(cand[:], top[:, 0:8], cand[:], NEG_INF)
                nc.vector.max(top[:, 8:16], cand[:])
                nc.vector.match_replace(cand[:], top[:, 8:16], cand[:], NEG_INF)
                nc.vector.max(top[:, 16:24], cand[:])

                # transpose [128, 20] -> [20, 128] into feat
                pst = psum_t.tile([K, 128], f32, tag="pst")
                nc.tensor.transpose(pst[:], top[:, 0:K], identity[:])
                nc.any.tensor_copy(feat[:, rt * 128:(rt + 1) * 128], pst[:])

            # --- MLP stack: feat [20, n] -> h1 [256, n] -> h2 [128, n] -> [1, n] ---
            relu = mybir.ActivationFunctionType.Relu
            sigm = mybir.ActivationFunctionType.Sigmoid
            for j in range(2):
                for q in range(ROWS_PER_CORE // CHUNK):
                    ps = psum_pd.tile([128, CHUNK], f32, tag="pd")
                    nc.tensor.matmul(
                        ps[:], w1t[:, j * 128:(j + 1) * 128],
                        feat[:, q * CHUNK:(q + 1) * CHUNK],
                        start=True, stop=True,
                    )
                    nc.scalar.activation(
                        h1[:, j, q * CHUNK:(q + 1) * CHUNK], ps[:], relu,
                        bias=b1[:, j:j + 1],
                    )
            for q in range(ROWS_PER_CORE // CHUNK):
                ps = psum_pd.tile([128, CHUNK], f32, tag="pd")
                nc.tensor.matmul(ps[:], w2t[:, 0, :], h1[:, 0, q * CHUNK:(q + 1) * CHUNK],
                                 start=True, stop=False)
                nc.tensor.matmul(ps[:], w2t[:, 1, :], h1[:, 1, q * CHUNK:(q + 1) * CHUNK],
                                 start=False, stop=True)
                nc.scalar.activation(
                    h2[:, q * CHUNK:(q + 1) * CHUNK], ps[:], relu, bias=b2[:, 0:1],
                )
            for q in range(ROWS_PER_CORE // CHUNK):
                po = psum_o.tile([1, CHUNK], f32, tag="po")
                nc.tensor.matmul(po[:], w3t[:], h2[:, q * CHUNK:(q + 1) * CHUNK],
                                 start=True, stop=True)
                nc.scalar.activation(out_sb[:, q * CHUNK:(q + 1) * CHUNK], po[:], sigm)

            nc.sync.dma_start(out_d[:], out_sb[:])

    nc.compile()
    return nc


def _prep_inputs(x, y, W1, gamma1, beta1, mean1, var1,
                 W2, gamma2, beta2, mean2, var2, W3):
    """Host-side prep: distance augmentation + BN folding. All O(N) small."""
    x = np.asarray(x, np.float32)
    y = np.asarray(y, np.float32)
    xx = (x * x).sum(-1)                         # [B, N]
    yy = (y * y).sum(-1)                         # [B, M]

    # pd = sum_k xaug[k,n] * yaug[k,m]
    xaug = np.zeros((B, KAUG, N), np.float32)
    xaug[:, 0:3] = x.transpose(0, 2, 1)
    xaug[:, 3] = xx
    xaug[:, 4] = 1.0
    yaug = np.zeros((B, KAUG, M), np.float32)
    yaug[:, 0:3] = 2.0 * y.transpose(0, 2, 1)
    yaug[:, 3] = -1.0
    yaug[:, 4] = -yy

    inv1 = np.asarray(gamma1, np.float32) / np.sqrt(np.asarray(var1, np.float32) + BN_EPS)
    w1e = (inv1[:, None] * np.asarray(W1, np.float32))          # [256, 20]
    b1 = np.asarray(beta1, np.float32) - np.asarray(mean1, np.float32) * inv1
    inv2 = np.asarray(gamma2, np.float32) / np.sqrt(np.asarray(var2, np.float32) + BN_EPS)
    w2e = (inv2[:, None] * np.asarray(W2, np.float32))          # [128, 256]
    b2 = np.asarray(beta2, np.float32) - np.asarray(mean2, np.float32) * inv2

    w1t = np.ascontiguousarray(w1e.T)                            # [20, 256]
    b1p = np.ascontiguousarray(b1.reshape(2, 128).T)             # [128, 2]
    w2t = np.ascontiguousarray(w2e.T.reshape(2, 128, 128).transpose(1, 0, 2))  # [128,2,128]
    b2p = np.ascontiguousarray(b2.reshape(128, 1))               # [128, 1]
    w3t = np.ascontiguousarray(np.asarray(W3, np.float32).T)     # [128, 1]

    in_maps = []
    for c in range(N_CORES):
        b = c // CORES_PER_BATCH
        r0 = (c % CORES_PER_BATCH) * ROWS_PER_CORE
        in_maps.append({
            "xaug": np.ascontiguousarray(xaug[b, :, r0:r0 + ROWS_PER_CORE]),
            "yaug": np.ascontiguousarray(yaug[b]),
            "w1t": w1t, "b1": b1p, "w2t": w2t, "b2": b2p, "w3t": w3t,
        })
    return in_maps


def kernel(x, y, W1, gamma1, beta1, mean1, var1,
           W2, gamma2, beta2, mean2, var2, W3, k, _trace=False):
    from concourse.bass_utils import run_bass_kernel_spmd

    assert int(k) == K
    if TOPK_MODE not in _CACHE:
        _CACHE[TOPK_MODE] = _build(TOPK_MODE)
    nc = _CACHE[TOPK_MODE]

    in_maps = _prep_inputs(x, y, W1, gamma1, beta1, mean1, var1,
                           W2, gamma2, beta2, mean2, var2, W3)
    res = run_bass_kernel_spmd(nc, in_maps, core_ids=list(range(N_CORES)),
                               trace=_trace)
    out = np.empty((B, N, 1), np.float32)
    for c in range(N_CORES):
        b = c // CORES_PER_BATCH
        r0 = (c % CORES_PER_BATCH) * ROWS_PER_CORE
        out[b, r0:r0 + ROWS_PER_CORE, 0] = res.results[c]["out"][0]
    kernel.last_result = res
    return out
